# revision 7
# baseline (speedup 1.0000x reference)
"""DeepONet (branch/trunk MLP + segment-sum) Bass kernel for 8 TRN2 NeuronCores.

Strategy: pure data parallel over the batch dim (131072 rows -> 16384/core),
MLP weights replicated. All activations live in a transposed layout
([features-on-partitions, rows-on-free]) so no transposes are needed between
layers; the input x tile is transposed once via the PE. The final segmented
feature reduction is a matmul with the one-hot segment-indicator matrix S,
accumulated in PSUM, so it handles arbitrary seg_ids. Matmuls run in fp32r
(full PE rate at N>=256, ~1e-3 precision).
"""
import numpy as np
from concourse import bacc, mybir, tile
from concourse.bass_utils import run_bass_kernel_spmd

N_CORES = 8
B = 131072
D_IN = 130
BR_IN = 128
TR_IN = 2
HID = 256
OUT = 1024
NCH = 8
R = B // N_CORES            # rows per core
TILE_ROWS = 512             # moving-operand free dim per matmul (fp32 max)
BLK = TILE_ROWS // 128      # 128-row blocks per row tile
NT = R // TILE_ROWS         # row tiles per core

F32R = mybir.dt.float32r
F32 = mybir.dt.float32
AF = mybir.ActivationFunctionType
ALU = mybir.AluOpType
ts = lambda i, n: slice(i * n, (i + 1) * n)

_NC_CACHE = {}


def _build(nt):
    nc = bacc.Bacc("TRN2", target_bir_lowering=False, debug=False)

    def din(name, shape, dt=F32R):
        return nc.dram_tensor(name, shape, dt, kind="ExternalInput").ap()

    rows = nt * TILE_ROWS
    xd = din("x", [nt, 128, BLK, D_IN])           # (tile, p, blk, feat)
    bw0 = din("bw0", [128, 2, 128])               # branch L1: [K, mchunk, 128]
    bw1 = din("bw1", [128, 2, HID])               # [p, kchunk, M]
    bw2 = din("bw2", [128, 2, HID])
    bw3 = din("bw3", [128, 2, OUT])
    tw0 = din("tw0", [TR_IN, 2, 128])             # trunk L1: K=2
    tw1 = din("tw1", [128, 2, HID])
    tw2 = din("tw2", [128, 2, HID])
    tw3 = din("tw3", [128, 2, OUT])
    bb0 = din("bb0", [128, 2], F32)               # biases, per-partition chunks
    bb1 = din("bb1", [128, 2], F32)
    bb2 = din("bb2", [128, 2], F32)
    bb3 = din("bb3", [128, NCH], F32)
    tb0 = din("tb0", [128, 2], F32)
    tb1 = din("tb1", [128, 2], F32)
    tb2 = din("tb2", [128, 2], F32)
    tb3 = din("tb3", [128, NCH], F32)
    sd = din("S", [128, NCH, NCH])                # one-hot seg matrix chunks
    idd = din("ident", [128, 128])
    out_t = nc.dram_tensor("out_t", [NCH, rows], F32, kind="ExternalOutput").ap()

    with tile.TileContext(nc) as tc:
        with (
            tc.tile_pool(name="wpool", bufs=1) as wp,
            tc.tile_pool(name="xpool", bufs=3) as xp,
            tc.tile_pool(name="apool", bufs=2) as ap,
            tc.tile_pool(name="ptr", bufs=2, space="PSUM") as ptr,
            tc.tile_pool(name="pmm", bufs=4, space="PSUM") as pmm,
            tc.tile_pool(name="pseg", bufs=2, space="PSUM") as pseg,
        ):
            # --- load weights/constants once ---
            def wload(apx, shape, dt=F32R, tag=None):
                t = wp.tile(shape, dt, tag=tag)
                nc.sync.dma_start(t[:], apx[:])
                return t

            bw0_s = wload(bw0, [128, 2, 128], tag="bw0")
            bw1_s = wload(bw1, [128, 2, HID], tag="bw1")
            bw2_s = wload(bw2, [128, 2, HID], tag="bw2")
            bw3_s = wload(bw3, [128, 2, OUT], tag="bw3")
            tw0_s = wload(tw0, [TR_IN, 2, 128], tag="tw0")
            tw1_s = wload(tw1, [128, 2, HID], tag="tw1")
            tw2_s = wload(tw2, [128, 2, HID], tag="tw2")
            tw3_s = wload(tw3, [128, 2, OUT], tag="tw3")
            bb0_s = wload(bb0, [128, 2], F32, tag="bb0")
            bb1_s = wload(bb1, [128, 2], F32, tag="bb1")
            bb2_s = wload(bb2, [128, 2], F32, tag="bb2")
            bb3_s = wload(bb3, [128, NCH], F32, tag="bb3")
            tb0_s = wload(tb0, [128, 2], F32, tag="tb0")
            tb1_s = wload(tb1, [128, 2], F32, tag="tb1")
            tb2_s = wload(tb2, [128, 2], F32, tag="tb2")
            tb3_s = wload(tb3, [128, NCH], F32, tag="tb3")
            s_s = wload(sd, [128, NCH, NCH], tag="S")
            id_s = wload(idd, [128, 128], tag="ident")

            bw = [bw1_s, bw2_s]
            tw = [tw1_s, tw2_s]
            bb = [bb1_s, bb2_s]
            tb = [tb1_s, tb2_s]

            for r in range(nt):
                # --- load + transpose input tile ---
                xt = xp.tile([128, BLK, D_IN], F32R, tag="xt")
                nc.sync.dma_start(xt[:], xd[r])
                x1t = ap.tile([128, TILE_ROWS], F32R, tag="x1t")
                x2t = ap.tile([TR_IN, TILE_ROWS], F32R, tag="x2t")
                for j in range(BLK):
                    p1 = ptr.tile([128, 128], F32R, tag="ptr")
                    nc.tensor.transpose(p1[:], xt[:, j, 0:BR_IN], id_s[:])
                    nc.vector.tensor_copy(x1t[:, ts(j, 128)], p1[:])
                    p2 = ptr.tile([128, 128], F32R, tag="ptr")
                    nc.tensor.transpose(p2[0:TR_IN, :], xt[:, j, BR_IN:D_IN], id_s[:])
                    nc.vector.tensor_copy(x2t[:, ts(j, 128)], p2[0:TR_IN, :])

                # --- layer 1 (branch K=128, trunk K=2) ---
                h_b = ap.tile([128, 2, TILE_ROWS], F32R, tag="hb1")
                h_t = ap.tile([128, 2, TILE_ROWS], F32R, tag="ht1")
                for m in range(2):
                    pb = pmm.tile([128, TILE_ROWS], F32, tag="mm")
                    nc.tensor.matmul(pb[:], bw0_s[:, m, :], x1t[:],
                                     start=True, stop=True)
                    nc.scalar.activation(h_b[:, m, :], pb[:], AF.Tanh,
                                         bias=bb0_s[:, m:m + 1])
                    pt = pmm.tile([128, TILE_ROWS], F32, tag="mm")
                    nc.tensor.matmul(pt[:], tw0_s[:, m, :], x2t[:],
                                     start=True, stop=True)
                    nc.scalar.activation(h_t[:, m, :], pt[:], AF.Tanh,
                                         bias=tb0_s[:, m:m + 1])

                # --- layers 2..3 (K=256 -> 2 psum-accumulated matmuls) ---
                for li in range(2):
                    nh_b = ap.tile([128, 2, TILE_ROWS], F32R, tag=f"hb{li + 2}")
                    nh_t = ap.tile([128, 2, TILE_ROWS], F32R, tag=f"ht{li + 2}")
                    for m in range(2):
                        pb = pmm.tile([128, TILE_ROWS], F32, tag="mm")
                        nc.tensor.matmul(pb[:], bw[li][:, 0, ts(m, 128)],
                                         h_b[:, 0, :], start=True, stop=False)
                        nc.tensor.matmul(pb[:], bw[li][:, 1, ts(m, 128)],
                                         h_b[:, 1, :], start=False, stop=True)
                        nc.scalar.activation(nh_b[:, m, :], pb[:], AF.Tanh,
                                             bias=bb[li][:, m:m + 1])
                        pt = pmm.tile([128, TILE_ROWS], F32, tag="mm")
                        nc.tensor.matmul(pt[:], tw[li][:, 0, ts(m, 128)],
                                         h_t[:, 0, :], start=True, stop=False)
                        nc.tensor.matmul(pt[:], tw[li][:, 1, ts(m, 128)],
                                         h_t[:, 1, :], start=False, stop=True)
                        nc.scalar.activation(nh_t[:, m, :], pt[:], AF.Tanh,
                                             bias=tb[li][:, m:m + 1])
                    h_b, h_t = nh_b, nh_t

                # --- layer 4 + product + segment reduce ---
                ps_out = pseg.tile([NCH, TILE_ROWS], F32, tag="pseg")
                pending = []
                for m in range(NCH):
                    p1 = pmm.tile([128, TILE_ROWS], F32, tag="mm")
                    nc.tensor.matmul(p1[:], bw3_s[:, 0, ts(m, 128)], h_b[:, 0, :],
                                     start=True, stop=False)
                    nc.tensor.matmul(p1[:], bw3_s[:, 1, ts(m, 128)], h_b[:, 1, :],
                                     start=False, stop=True)
                    p2 = pmm.tile([128, TILE_ROWS], F32, tag="mm")
                    nc.tensor.matmul(p2[:], tw3_s[:, 0, ts(m, 128)], h_t[:, 0, :],
                                     start=True, stop=False)
                    nc.tensor.matmul(p2[:], tw3_s[:, 1, ts(m, 128)], h_t[:, 1, :],
                                     start=False, stop=True)
                    o1 = ap.tile([128, TILE_ROWS], F32, tag="o1")
                    nc.scalar.activation(o1[:], p1[:], AF.Identity,
                                         bias=bb3_s[:, m:m + 1])
                    pr = ap.tile([128, TILE_ROWS], F32R, tag="prod")
                    nc.vector.scalar_tensor_tensor(
                        pr[:], p2[:], tb3_s[:, m:m + 1], o1[:],
                        op0=ALU.add, op1=ALU.mult)
                    pending.append((m, pr))
                    if len(pending) >= 2:
                        mi, pri = pending.pop(0)
                        nc.tensor.matmul(ps_out[:], s_s[:, mi, :], pri[:],
                                         start=(mi == 0), stop=False,
                                         skip_group_check=True)
                for mi, pri in pending:
                    nc.tensor.matmul(ps_out[:], s_s[:, mi, :], pri[:],
                                     start=(mi == 0), stop=(mi == NCH - 1),
                                     skip_group_check=True)
                ot = ap.tile([NCH, TILE_ROWS], F32, tag="ot")
                nc.vector.tensor_copy(ot[:], ps_out[:])
                nc.sync.dma_start(out_t[:, ts(r, TILE_ROWS)], ot[:])

    nc.compile()
    return nc


def _prep_shared(br_Ws, br_bs, tr_Ws, tr_bs, seg_ids):
    f = np.float32
    d = {}
    d["bw0"] = np.ascontiguousarray(
        np.asarray(br_Ws[0], f).reshape(128, 2, 128))
    d["tw0"] = np.ascontiguousarray(
        np.asarray(tr_Ws[0], f).reshape(TR_IN, 2, 128))
    for nm, W in (("bw1", br_Ws[1]), ("bw2", br_Ws[2]), ("bw3", br_Ws[3]),
                  ("tw1", tr_Ws[1]), ("tw2", tr_Ws[2]), ("tw3", tr_Ws[3])):
        w = np.asarray(W, f)
        d[nm] = np.ascontiguousarray(
            w.reshape(2, 128, w.shape[1]).transpose(1, 0, 2))
    for nm, b in (("bb0", br_bs[0]), ("bb1", br_bs[1]), ("bb2", br_bs[2]),
                  ("tb0", tr_bs[0]), ("tb1", tr_bs[1]), ("tb2", tr_bs[2])):
        d[nm] = np.ascontiguousarray(np.asarray(b, f).reshape(2, 128).T)
    d["bb3"] = np.ascontiguousarray(np.asarray(br_bs[3], f).reshape(NCH, 128).T)
    d["tb3"] = np.ascontiguousarray(np.asarray(tr_bs[3], f).reshape(NCH, 128).T)
    seg = np.asarray(seg_ids).astype(np.int64)
    S = (seg[:, None] == np.arange(NCH)[None, :]).astype(f)      # [1024, 8]
    d["S"] = np.ascontiguousarray(S.reshape(NCH, 128, NCH).transpose(1, 0, 2))
    d["ident"] = np.eye(128, dtype=f)
    return d


def kernel(x, br_Ws, br_bs, tr_Ws, tr_bs, seg_ids):
    x = np.asarray(x, np.float32)
    assert x.shape == (B, D_IN)
    shared = _prep_shared(br_Ws, br_bs, tr_Ws, tr_bs, seg_ids)

    if NT not in _NC_CACHE:
        _NC_CACHE[NT] = _build(NT)
    nc = _NC_CACHE[NT]

    in_maps = []
    for c in range(N_CORES):
        xs = x[c * R:(c + 1) * R]                      # [R, 130]
        xr = np.ascontiguousarray(
            xs.reshape(NT, BLK, 128, D_IN).transpose(0, 2, 1, 3))
        in_maps.append({"x": xr, **shared})
    res = run_bass_kernel_spmd(nc, in_maps, list(range(N_CORES)))
    out = np.concatenate(
        [np.asarray(res.results[c]["out_t"]).T for c in range(N_CORES)], axis=0)
    return out.astype(np.float32)


# revision 18
# speedup vs baseline: 1.0548x; 1.0548x over previous
"""DeepONet (branch/trunk MLP + segment-sum) Bass kernel for 8 TRN2 NeuronCores.

Strategy: pure data parallel over the batch dim (131072 rows -> 16384/core),
MLP weights replicated. All activations live in a transposed layout
([features-on-partitions, rows-on-free]); x is pre-transposed on the host so
no on-device transposes are needed. The final segmented feature reduction is
a matmul with the one-hot segment-indicator matrix S accumulated in PSUM, so
it handles arbitrary seg_ids. Matmuls run in fp32r (full PE rate at N>=256,
~1e-3 precision). Output is produced channel-major [8, rows] and transposed
back on the host.
"""
import os
import numpy as np
from concourse import bacc, mybir, tile
from concourse.bass_utils import run_bass_kernel_spmd

N_CORES = 8
B = 131072
D_IN = 130
BR_IN = 128
TR_IN = 2
HID = 256
OUT = 1024
NCH = 8
R = B // N_CORES            # rows per core
TILE_ROWS = 512             # moving-operand free dim per matmul (fp32 max)
NT = R // TILE_ROWS         # row tiles per core

F32R = mybir.dt.float32r
F32 = mybir.dt.float32
AF = mybir.ActivationFunctionType
ALU = mybir.AluOpType
ts = lambda i, n: slice(i * n, (i + 1) * n)

# how many of the 8 branch-bias adds go to the Scalar engine (rest on DVE)
N_IDENT_ACT = int(os.environ.get("K_IDENT_ACT", "4"))
# col-tiled segment reduce (4 concurrent MMs into col groups + PE gather)
SEG_PACK = os.environ.get("K_SEG_PACK", "0") == "1"
# row-packed trunk L1 (both m-chunks concurrent in row groups 0/1)
TRUNK_PACK = os.environ.get("K_TRUNK_PACK", "0") == "1"

_NC_CACHE = {}


def _build(nt):
    nc = bacc.Bacc("TRN2", target_bir_lowering=False, debug=False)

    def din(name, shape, dt=F32R):
        return nc.dram_tensor(name, shape, dt, kind="ExternalInput").ap()

    rows = nt * TILE_ROWS
    xd = din("xt", [D_IN, rows])                  # pre-transposed input
    bw0 = din("bw0", [128, 2, 128])               # branch L1: [K, mchunk, 128]
    bw1 = din("bw1", [128, 2, HID])               # [p, kchunk, M]
    bw2 = din("bw2", [128, 2, HID])
    bw3 = din("bw3", [128, 2, OUT])
    tw0 = din("tw0", [TR_IN, 2, 128])             # trunk L1: K=2
    tw1 = din("tw1", [128, 2, HID])
    tw2 = din("tw2", [128, 2, HID])
    tw3 = din("tw3", [128, 2, OUT])
    bb0 = din("bb0", [128, 2], F32)               # biases, per-partition chunks
    bb1 = din("bb1", [128, 2], F32)
    bb2 = din("bb2", [128, 2], F32)
    bb3 = din("bb3", [128, NCH], F32)
    tb0 = din("tb0", [128, 2], F32)
    tb1 = din("tb1", [128, 2], F32)
    tb2 = din("tb2", [128, 2], F32)
    tb3 = din("tb3", [128, NCH], F32)
    sd = din("S", [128, NCH, NCH])                # one-hot seg matrix chunks
    if SEG_PACK:
        gd = din("G", [128, NCH])                 # col-group gather matrix
    out_t = nc.dram_tensor("out_t", [NCH, rows], F32, kind="ExternalOutput").ap()

    with tile.TileContext(nc) as tc:
        with (
            tc.tile_pool(name="wpool", bufs=1) as wp,
            tc.tile_pool(name="apool", bufs=2) as ap,
            tc.tile_pool(name="xpool", bufs=3) as xp,
            tc.tile_pool(name="pmm", bufs=(4 if SEG_PACK else 6),
                         space="PSUM") as pmm,
            tc.tile_pool(name="pseg", bufs=2, space="PSUM") as pseg,
        ):
            # --- load weights/constants once ---
            def wload(apx, shape, dt=F32R, tag=None):
                t = wp.tile(shape, dt, tag=tag)
                nc.sync.dma_start(t[:], apx[:])
                return t

            if SEG_PACK:
                g_s = wload(gd, [128, NCH], tag="G")
            bw0_s = wload(bw0, [128, 2, 128], tag="bw0")
            bw1_s = wload(bw1, [128, 2, HID], tag="bw1")
            bw2_s = wload(bw2, [128, 2, HID], tag="bw2")
            bw3_s = wload(bw3, [128, 2, OUT], tag="bw3")
            if TRUNK_PACK:
                tw0_s = wp.tile([34, 2, 128], F32R, tag="tw0")
                nc.sync.dma_start(tw0_s[0:TR_IN], tw0[:])
                nc.sync.dma_start(tw0_s[32:32 + TR_IN], tw0[:])
            else:
                tw0_s = wload(tw0, [TR_IN, 2, 128], tag="tw0")
            tw1_s = wload(tw1, [128, 2, HID], tag="tw1")
            tw2_s = wload(tw2, [128, 2, HID], tag="tw2")
            tw3_s = wload(tw3, [128, 2, OUT], tag="tw3")
            bb0_s = wload(bb0, [128, 2], F32, tag="bb0")
            bb1_s = wload(bb1, [128, 2], F32, tag="bb1")
            bb2_s = wload(bb2, [128, 2], F32, tag="bb2")
            bb3_s = wload(bb3, [128, NCH], F32, tag="bb3")
            tb0_s = wload(tb0, [128, 2], F32, tag="tb0")
            tb1_s = wload(tb1, [128, 2], F32, tag="tb1")
            tb2_s = wload(tb2, [128, 2], F32, tag="tb2")
            tb3_s = wload(tb3, [128, NCH], F32, tag="tb3")
            s_s = wload(sd, [128, NCH, NCH], tag="S")

            bw = [bw1_s, bw2_s]
            tw = [tw1_s, tw2_s]
            bb = [bb1_s, bb2_s]
            tb = [tb1_s, tb2_s]

            for r in range(nt):
                # --- load input tile (already transposed) ---
                x1t = xp.tile([128, TILE_ROWS], F32R, tag="x1t")
                nc.sync.dma_start(x1t[:], xd[0:BR_IN, ts(r, TILE_ROWS)])
                if TRUNK_PACK:
                    x2t = xp.tile([34, TILE_ROWS], F32R, tag="x2t")
                    nc.sync.dma_start(x2t[0:TR_IN, :],
                                      xd[BR_IN:D_IN, ts(r, TILE_ROWS)])
                    nc.sync.dma_start(x2t[32:32 + TR_IN, :],
                                      xd[BR_IN:D_IN, ts(r, TILE_ROWS)])
                else:
                    x2t = xp.tile([TR_IN, TILE_ROWS], F32R, tag="x2t")
                    nc.sync.dma_start(x2t[:], xd[BR_IN:D_IN, ts(r, TILE_ROWS)])

                # --- layer 1 (branch K=128, trunk K=2) ---
                h_b = ap.tile([128, 2, TILE_ROWS], F32R, tag="hb1")
                h_t = ap.tile([128, 2, TILE_ROWS], F32R, tag="ht1")
                for m in range(2):
                    pb = pmm.tile([128, TILE_ROWS], F32, tag="mm")
                    nc.tensor.matmul(pb[:], bw0_s[:, m, :], x1t[:],
                                     start=True, stop=True)
                    nc.scalar.activation(h_b[:, m, :], pb[:], AF.Tanh,
                                         bias=bb0_s[:, m:m + 1])
                    pt = pmm.tile([128, TILE_ROWS], F32, tag="mm")
                    if TRUNK_PACK:
                        base = 32 * m
                        nc.tensor.matmul(pt[:], tw0_s[base:base + TR_IN, m, :],
                                         x2t[base:base + TR_IN, :],
                                         start=True, stop=True,
                                         tile_position=(base, 0))
                    else:
                        nc.tensor.matmul(pt[:], tw0_s[:, m, :], x2t[:],
                                         start=True, stop=True)
                    nc.scalar.activation(h_t[:, m, :], pt[:], AF.Tanh,
                                         bias=tb0_s[:, m:m + 1])

                # --- layers 2..3 (K=256 -> 2 psum-accumulated matmuls) ---
                for li in range(2):
                    nh_b = ap.tile([128, 2, TILE_ROWS], F32R, tag=f"hb{li + 2}")
                    nh_t = ap.tile([128, 2, TILE_ROWS], F32R, tag=f"ht{li + 2}")
                    for m in range(2):
                        pb = pmm.tile([128, TILE_ROWS], F32, tag="mm")
                        nc.tensor.matmul(pb[:], bw[li][:, 0, ts(m, 128)],
                                         h_b[:, 0, :], start=True, stop=False)
                        nc.tensor.matmul(pb[:], bw[li][:, 1, ts(m, 128)],
                                         h_b[:, 1, :], start=False, stop=True)
                        nc.scalar.activation(nh_b[:, m, :], pb[:], AF.Tanh,
                                             bias=bb[li][:, m:m + 1])
                        pt = pmm.tile([128, TILE_ROWS], F32, tag="mm")
                        nc.tensor.matmul(pt[:], tw[li][:, 0, ts(m, 128)],
                                         h_t[:, 0, :], start=True, stop=False)
                        nc.tensor.matmul(pt[:], tw[li][:, 1, ts(m, 128)],
                                         h_t[:, 1, :], start=False, stop=True)
                        nc.scalar.activation(nh_t[:, m, :], pt[:], AF.Tanh,
                                             bias=tb[li][:, m:m + 1])
                    h_b, h_t = nh_b, nh_t

                # --- layer 4 + product + segment reduce ---
                if SEG_PACK:
                    # 4-col-group packed partial sums, combined by a PE gather
                    ps_seg = pseg.tile([128, TILE_ROWS], F32, tag="pseg")
                else:
                    ps_out = pseg.tile([NCH, TILE_ROWS], F32, tag="pseg")
                pending = []
                for m in range(NCH):
                    p1 = pmm.tile([128, TILE_ROWS], F32, tag="mm")
                    nc.tensor.matmul(p1[:], bw3_s[:, 0, ts(m, 128)], h_b[:, 0, :],
                                     start=True, stop=False)
                    nc.tensor.matmul(p1[:], bw3_s[:, 1, ts(m, 128)], h_b[:, 1, :],
                                     start=False, stop=True)
                    p2 = pmm.tile([128, TILE_ROWS], F32, tag="mm")
                    nc.tensor.matmul(p2[:], tw3_s[:, 0, ts(m, 128)], h_t[:, 0, :],
                                     start=True, stop=False)
                    nc.tensor.matmul(p2[:], tw3_s[:, 1, ts(m, 128)], h_t[:, 1, :],
                                     start=False, stop=True)
                    o1 = ap.tile([128, TILE_ROWS], F32, tag="o1")
                    if m < N_IDENT_ACT:
                        nc.scalar.activation(o1[:], p1[:], AF.Identity,
                                             bias=bb3_s[:, m:m + 1])
                    else:
                        nc.vector.tensor_scalar_add(o1[:], p1[:],
                                                    bb3_s[:, m:m + 1])
                    pr = ap.tile([128, TILE_ROWS], F32R, tag="prod")
                    nc.vector.scalar_tensor_tensor(
                        pr[:], p2[:], tb3_s[:, m:m + 1], o1[:],
                        op0=ALU.add, op1=ALU.mult)
                    pending.append((m, pr))

                    def seg_mm(mi, pri):
                        if SEG_PACK:
                            base = 32 * (mi % 4)
                            nc.tensor.matmul(
                                ps_seg[base:base + NCH, :], s_s[:, mi, :],
                                pri[:], start=(mi < 4), stop=(mi >= 4),
                                tile_position=(0, base),
                                skip_group_check=True)
                        else:
                            nc.tensor.matmul(
                                ps_out[:], s_s[:, mi, :], pri[:],
                                start=(mi == 0), stop=(mi == NCH - 1),
                                skip_group_check=True)

                    if len(pending) >= 2:
                        seg_mm(*pending.pop(0))
                for mi_pri in pending:
                    seg_mm(*mi_pri)
                if SEG_PACK:
                    prs = ap.tile([128, TILE_ROWS], F32R, tag="prs")
                    nc.vector.tensor_copy(prs[:], ps_seg[:])
                    ps_out = pseg.tile([NCH, TILE_ROWS], F32, tag="pout")
                    nc.tensor.matmul(ps_out[:], g_s[:], prs[:],
                                     start=True, stop=True)
                ot = ap.tile([NCH, TILE_ROWS], F32, tag="ot")
                nc.vector.tensor_copy(ot[:], ps_out[:])
                nc.sync.dma_start(out_t[:, ts(r, TILE_ROWS)], ot[:])

    nc.compile()
    return nc


def _prep_shared(br_Ws, br_bs, tr_Ws, tr_bs, seg_ids):
    f = np.float32
    d = {}
    d["bw0"] = np.ascontiguousarray(
        np.asarray(br_Ws[0], f).reshape(128, 2, 128))
    d["tw0"] = np.ascontiguousarray(
        np.asarray(tr_Ws[0], f).reshape(TR_IN, 2, 128))
    for nm, W in (("bw1", br_Ws[1]), ("bw2", br_Ws[2]), ("bw3", br_Ws[3]),
                  ("tw1", tr_Ws[1]), ("tw2", tr_Ws[2]), ("tw3", tr_Ws[3])):
        w = np.asarray(W, f)
        d[nm] = np.ascontiguousarray(
            w.reshape(2, 128, w.shape[1]).transpose(1, 0, 2))
    for nm, b in (("bb0", br_bs[0]), ("bb1", br_bs[1]), ("bb2", br_bs[2]),
                  ("tb0", tr_bs[0]), ("tb1", tr_bs[1]), ("tb2", tr_bs[2])):
        d[nm] = np.ascontiguousarray(np.asarray(b, f).reshape(2, 128).T)
    d["bb3"] = np.ascontiguousarray(np.asarray(br_bs[3], f).reshape(NCH, 128).T)
    d["tb3"] = np.ascontiguousarray(np.asarray(tr_bs[3], f).reshape(NCH, 128).T)
    seg = np.asarray(seg_ids).astype(np.int64)
    S = (seg[:, None] == np.arange(NCH)[None, :]).astype(f)      # [1024, 8]
    d["S"] = np.ascontiguousarray(S.reshape(NCH, 128, NCH).transpose(1, 0, 2))
    G = np.zeros((128, NCH), f)
    for j in range(4):
        for c in range(NCH):
            G[32 * j + c, c] = 1.0
    d["G"] = G
    return d


def _x_maps(x, nt=NT):
    """Per-core pre-transposed x shards: [130, nt*TILE_ROWS] each."""
    rows = nt * TILE_ROWS
    xt = np.asarray(x, np.float32).T               # [130, B]
    return [np.ascontiguousarray(xt[:, c * R: c * R + rows])
            for c in range(N_CORES)]


def kernel(x, br_Ws, br_bs, tr_Ws, tr_bs, seg_ids):
    x = np.asarray(x, np.float32)
    assert x.shape == (B, D_IN)
    shared = _prep_shared(br_Ws, br_bs, tr_Ws, tr_bs, seg_ids)

    if NT not in _NC_CACHE:
        _NC_CACHE[NT] = _build(NT)
    nc = _NC_CACHE[NT]

    in_maps = [{"xt": xs, **shared} for xs in _x_maps(x)]
    res = run_bass_kernel_spmd(nc, in_maps, list(range(N_CORES)))
    out = np.concatenate(
        [np.asarray(res.results[c]["out_t"]).T for c in range(N_CORES)], axis=0)
    return out.astype(np.float32)
